# revision 33
# baseline (speedup 1.0000x reference)
"""Distributed attention kernel for Trainium2 (8 NeuronCores).

Sharding: B*H = 2*16 = 32 (batch, head) pairs over 8 cores.
Core c handles batch b = c//4 and global heads 4*(c%4) .. 4*(c%4)+3
(i.e. output columns (c%4)*256 : (c%4+1)*256 of the 1024-wide output).

Per-core kernel (compute in bf16, f32 PSUM accumulation):
  - inputs arrive pre-transposed from host: qT/kT/vT [1024, 2048] bf16,
    weight column slices wq/wk/wv [1024, 256] bf16, mask columns msk
    [128, 16] f32 (0/1), identity [128, 128] f32.
  - projections: QWT/KWT in [d, s] layout, VW in [s, d] layout. VW gets a
    mask-valued ones column appended per head (65 cols) so the PV matmul
    also produces sum(exp) in its last output row; VW rows for masked keys
    are zeroed, which implements the additive -1e12 key mask exactly.
  - scores computed transposed: S_T[k, q] so softmax needs no P transpose;
    the two heads of a projection tile are row-packed (64x128 PE tiling);
    exp on ScalarE over 3-bank PSUM tiles; PV accumulation into [65, 512].
  - epilogue: TensorE transposes [65,128]->[128,65] (O plus sumexp),
    reciprocal + per-partition scale on VectorE, DMA out (t-major layout,
    host reassembles).
"""

import numpy as np

HEADS = 16
DK = 64
DM = 1024
B = 2
S = 2048
HL = 4           # heads per core
NCOL = HL * DK   # 256 projection cols per core
NM = DM // 128   # 8 m-chunks
NKC = S // 128   # 16 k-chunks
NQC = S // 512   # 4 q-chunks
NSUB = 512 // 128
NBLK = 2 * NKC   # 32 score blocks of 512 per (t, qc); 2 blocks per s-tile
NST = NBLK // 2  # 16 s-tiles per (t, qc)

_CACHE = {}
CFG = {"pack_s": True, "pe_chain": False, "crit": False, "round_edge": False}


def _build(loop_n=None):
    from contextlib import ExitStack, nullcontext
    import concourse.bass as bass  # noqa: F401
    import concourse.mybir as mybir
    import concourse.bacc as bacc
    import concourse.tile as tile

    f32 = mybir.dt.float32
    bf16 = mybir.dt.bfloat16
    Exp = mybir.ActivationFunctionType.Exp

    nc = bacc.Bacc("TRN2", target_bir_lowering=False, debug=False, num_devices=8)

    qT = nc.dram_tensor("qT", [DM, S], bf16, kind="ExternalInput").ap()
    kT = nc.dram_tensor("kT", [DM, S], bf16, kind="ExternalInput").ap()
    vT = nc.dram_tensor("vT", [DM, S], bf16, kind="ExternalInput").ap()
    wq = nc.dram_tensor("wq", [DM, NCOL], bf16, kind="ExternalInput").ap()
    wk = nc.dram_tensor("wk", [DM, NCOL], bf16, kind="ExternalInput").ap()
    wv = nc.dram_tensor("wv", [DM, NCOL], bf16, kind="ExternalInput").ap()
    msk = nc.dram_tensor("msk", [128, NKC], f32, kind="ExternalInput").ap()
    ident = nc.dram_tensor("ident", [128, 128], f32, kind="ExternalInput").ap()
    # t-major output: rows [t*2048 + q], 128 cols (heads 2t, 2t+1)
    out = nc.dram_tensor("out", [2 * S, 128], f32, kind="ExternalOutput").ap()

    with tile.TileContext(nc) as tc, ExitStack() as ctx:
        const = ctx.enter_context(tc.tile_pool(name="const", bufs=1))
        bigp = ctx.enter_context(tc.tile_pool(name="bigp", bufs=2, space="PSUM"))
        op = ctx.enter_context(tc.tile_pool(name="op", bufs=4, space="PSUM"))
        ep = ctx.enter_context(tc.tile_pool(name="ep", bufs=24))
        otsp = ctx.enter_context(tc.tile_pool(name="otsp", bufs=4))
        outp = ctx.enter_context(tc.tile_pool(name="outp", bufs=4))
        rcp = ctx.enter_context(tc.tile_pool(name="rcp", bufs=4))

        # ---- persistent SBUF tensors ----
        xq = const.tile([128, NM * S], bf16, tag="xq")
        xk = const.tile([128, NM * S], bf16, tag="xk")
        xv = const.tile([128, NM * S], bf16, tag="xv")
        wq_sb = const.tile([128, NM * NCOL], bf16, tag="wq")
        wk_sb = const.tile([128, NM * NCOL], bf16, tag="wk")
        wv_sb = const.tile([128, NM * NCOL], bf16, tag="wv")
        m_sb = const.tile([128, NKC], f32, tag="m")
        id_sb = const.tile([128, 128], f32, tag="id")
        qwt = const.tile([128, 2 * S], bf16, tag="qwt")    # [d(2 heads), s] x2
        kwt = const.tile([128, 2 * S], bf16, tag="kwt")
        vw1 = const.tile([128, NKC * HL * 65], bf16, tag="vw1")

        if loop_n:
            # benchmark variant: run the whole body loop_n times on-device
            ctx.enter_context(tc.For_i(0, loop_n, 1))

        # ---- input DMA: weights/consts, then xk/xq interleaved, then xv ----
        for dst, src in ((wk_sb, wk), (wq_sb, wq), (wv_sb, wv)):
            nc.sync.dma_start(
                out=dst[:, :].rearrange("p (n d) -> p n d", n=NM),
                in_=src.rearrange("(n p) d -> p n d", p=128),
            )
        nc.sync.dma_start(out=m_sb[:, :], in_=msk)
        nc.sync.dma_start(out=id_sb[:, :], in_=ident)
        for m in range(NM):
            nc.sync.dma_start(
                out=xk[:, m * S: (m + 1) * S], in_=kT[m * 128: (m + 1) * 128, :]
            )
            nc.sync.dma_start(
                out=xq[:, m * S: (m + 1) * S], in_=qT[m * 128: (m + 1) * 128, :]
            )
        for m in range(NM):
            nc.sync.dma_start(
                out=xv[:, m * S: (m + 1) * S], in_=vT[m * 128: (m + 1) * 128, :]
            )

        # ---- ones columns of vw1 (mask-valued): vw1[:, kc, h, 64] = msk[:, kc]
        vw1_4d = vw1[:, :].rearrange("p (k h c) -> p k h c", k=NKC, h=HL)
        for h in range(HL):
            nc.vector.tensor_copy(vw1_4d[:, :, h, 64], m_sb[:, :])

        from concourse.tile_rust import add_dep_helper

        pe_prev = [None]
        last_s = [None]
        last_pv = [None]

        def pe_dep(bi):
            """Optionally chain TensorE instructions in emission order so the
            scheduler keeps same-PE-mode runs together (mode switches between
            64x128-tiled scores and default matmuls cost a PE drain)."""
            if CFG.get("pe_chain") and pe_prev[0] is not None:
                add_dep_helper(bi.ins, pe_prev[0].ins, sync=False,
                               reason="pe emission order")
            pe_prev[0] = bi
            return bi

        def proj_qk(w_sb, x_sb, dst, t, qc):
            ps = bigp.tile([128, 512], f32, tag="big", name=f"pqk{t}_{qc}")
            for m in range(NM):
                pe_dep(nc.tensor.matmul(
                    ps[:, :],
                    lhsT=w_sb[:, m * NCOL + t * 128: m * NCOL + t * 128 + 128],
                    rhs=x_sb[:, m * S + qc * 512: m * S + qc * 512 + 512],
                    start=(m == 0),
                    stop=(m == NM - 1),
                ))
            nc.vector.tensor_copy(
                dst[:, t * S + qc * 512: t * S + qc * 512 + 512], ps[:, :]
            )

        def proj_qk_mouter(w_sb, x_sb, dst, t):
            # m-outer: consume each x chunk as its DMA lands (4 live tiles)
            ps = [
                bigp.tile([128, 512], f32, tag="big", name=f"pm{t}_{q}")
                for q in (0, 1)
            ] + [
                op.tile([128, 512], f32, tag="o", name=f"pm{t}_{q}")
                for q in (2, 3)
            ]
            for m in range(NM):
                for qc in range(NQC):
                    pe_dep(nc.tensor.matmul(
                        ps[qc][:, :],
                        lhsT=w_sb[:, m * NCOL + t * 128: m * NCOL + t * 128 + 128],
                        rhs=x_sb[:, m * S + qc * 512: m * S + qc * 512 + 512],
                        start=(m == 0),
                        stop=(m == NM - 1),
                    ))
                    if m == NM - 1:
                        nc.vector.tensor_copy(
                            dst[:, t * S + qc * 512: t * S + qc * 512 + 512],
                            ps[qc][:, :],
                        )

        def proj_v(kb):
            ps = bigp.tile([128, NCOL], f32, tag="big", name=f"pv{kb}")
            for m in range(NM):
                pe_dep(nc.tensor.matmul(
                    ps[:, :],
                    lhsT=xv[:, m * S + kb * 128: m * S + kb * 128 + 128],
                    rhs=wv_sb[:, m * NCOL: (m + 1) * NCOL],
                    start=(m == 0),
                    stop=(m == NM - 1),
                ))
            nc.vector.tensor_scalar_mul(
                vw1_4d[:, kb, :, 0:64],
                ps[:, :].rearrange("p (h c) -> p h c", h=HL),
                m_sb[:, kb: kb + 1],
            )

        class Chunk:
            """Incremental emitter for one (t, qc) attention chunk."""

            def __init__(self, t, qc):
                self.t, self.qc = t, qc
                self.s_tiles = [None] * NST
                self.e_tiles = [None] * NST
                self.o_ps = None
                self.si = 0
                self.pi = 0

            def emit_s(self, n):
                t, qc = self.t, self.qc
                todo = list(range(self.si, min(self.si + n, NBLK)))
                if not todo:
                    return
                self.si = todo[-1] + 1
                for g in range(0, len(todo), 4):
                    blks = todo[g: g + 4]
                    for blk in blks:
                        st = blk // 2
                        if blk % 2 == 0:
                            self.s_tiles[st] = bigp.tile(
                                [128, 1024], f32, tag="big",
                                name=f"sps{t}_{qc}_{st}"
                            )
                    crit = tc.tile_critical() if CFG.get("crit") else None
                    if crit:
                        crit.__enter__()
                    for j, blk in enumerate(blks):
                        kc, a = divmod(blk, 2)
                        st, sc = divmod(blk, 2)
                        bi = pe_dep(nc.tensor.matmul(
                            self.s_tiles[st][:, sc * 512: (sc + 1) * 512],
                            lhsT=kwt[
                                64 * a: 64 * a + 64,
                                t * S + kc * 128: t * S + kc * 128 + 128,
                            ],
                            rhs=qwt[
                                64 * a: 64 * a + 64,
                                t * S + qc * 512: t * S + qc * 512 + 512,
                            ],
                            start=True,
                            stop=True,
                            tile_position=(64 * a, 0) if CFG["pack_s"] else None,
                        ))
                        if (CFG.get("round_edge") and j == 0
                                and last_pv[0] is not None):
                            add_dep_helper(bi.ins, last_pv[0].ins, sync=False,
                                           reason="round order s after pv")
                        last_s[0] = bi
                    if crit:
                        crit.__exit__(None, None, None)
                    for blk in blks:
                        st, sc = divmod(blk, 2)
                        if blk == NBLK - 1 or sc == 1:
                            w = (sc + 1) * 512
                            self.e_tiles[st] = ep.tile(
                                [128, 1024], bf16, tag="e",
                                name=f"et{t}_{qc}_{st}"
                            )
                            nc.scalar.activation(
                                self.e_tiles[st][:, 0:w],
                                self.s_tiles[st][:, 0:w],
                                Exp,
                                scale=0.125,
                            )

            def emit_pv(self, n):
                t, qc = self.t, self.qc
                if self.o_ps is None:
                    self.o_ps = [
                        op.tile([65, 512], f32, tag="o", name=f"ops{t}_{qc}_{a}")
                        for a in range(2)
                    ]
                blks = list(range(self.pi, min(self.pi + n, NBLK)))
                if not blks:
                    return
                self.pi = blks[-1] + 1
                crit = tc.tile_critical() if CFG.get("crit") else None
                if crit:
                    crit.__enter__()
                for j, blk in enumerate(blks):
                    kc, a = divmod(blk, 2)
                    st, sc = divmod(blk, 2)
                    bi = pe_dep(nc.tensor.matmul(
                        self.o_ps[a][:, :],
                        lhsT=vw1[
                            :, kc * HL * 65 + (2 * t + a) * 65:
                            kc * HL * 65 + (2 * t + a) * 65 + 65
                        ],
                        rhs=self.e_tiles[st][:, sc * 512: (sc + 1) * 512],
                        start=(kc == 0),
                        stop=(kc == NKC - 1),
                    ))
                    if (CFG.get("round_edge") and j == 0
                            and last_s[0] is not None):
                        add_dep_helper(bi.ins, last_s[0].ins, sync=False,
                                       reason="round order pv after s")
                    last_pv[0] = bi
                if crit:
                    crit.__exit__(None, None, None)

        def epilogue(t, qc, o_ps):
            # copy out, transpose O+sumexp, reciprocal, normalize, DMA out
            ots2 = []
            for a in range(2):
                ots = otsp.tile([65, 512], f32, tag="ots", name=f"ots{t}_{qc}_{a}")
                nc.vector.tensor_copy(ots[:, :], o_ps[a][:, :])
                ots2.append(ots)
            for sub in range(NSUB):
                t2 = op.tile([128, 2 * 65], f32, tag="o", name=f"t2{t}_{qc}_{sub}")
                for a in range(2):
                    pe_dep(nc.tensor.transpose(
                        t2[:, a * 65: (a + 1) * 65],
                        ots2[a][:, sub * 128: (sub + 1) * 128],
                        id_sb[0:65, 0:65],
                    ))
                rc = rcp.tile([128, 2], f32, tag="rc", name=f"rc{t}_{qc}_{sub}")
                t2_3d = t2[:, :].rearrange("p (h c) -> p h c", h=2)
                nc.vector.reciprocal_approx_fast(out=rc[:, :], in_=t2_3d[:, :, 64])
                o_out = outp.tile([128, 128], f32, tag="out", name=f"oo{t}_{qc}_{sub}")
                for a in range(2):
                    nc.vector.tensor_scalar_mul(
                        o_out[:, a * 64: (a + 1) * 64],
                        t2[:, a * 65: a * 65 + 64],
                        rc[:, a: a + 1],
                    )
                nc.sync.dma_start(
                    out=out[
                        t * S + qc * 512 + sub * 128:
                        t * S + qc * 512 + sub * 128 + 128, :
                    ],
                    in_=o_out[:, :],
                )

        # ---- schedule (PE order fully chained via pe_dep):
        # warmup: K/Q t0 projections woven with chunk0 scores; then rounds
        # of [scores x8, other-128-mode work, PV x8] so PE mode switches are
        # amortized over 8-matmul runs and ScalarE never starves.
        chunks = [Chunk(t, qc) for t in range(2) for qc in range(NQC)]

        proj_qk_mouter(wk_sb, xk, kwt, 0)
        proj_qk_mouter(wq_sb, xq, qwt, 0)
        chunks[0].emit_s(NBLK)
        # V projection + chunk1 scores + chunk0 PV
        for i in range(8):
            chunks[1].emit_s(4)
            proj_v(2 * i)
            proj_v(2 * i + 1)
            chunks[0].emit_pv(4)
        epilogue(0, 0, chunks[0].o_ps)
        # t1 projections + chunk2 scores + chunk1 PV
        projt1 = [(wk_sb, xk, kwt), (wq_sb, xq, qwt)]
        for j in range(8):
            w_sb, x_sb, dst = projt1[j // 4]
            chunks[2].emit_s(4)
            proj_qk(w_sb, x_sb, dst, 1, j % 4)
            chunks[1].emit_pv(4)
        epilogue(0, 1, chunks[1].o_ps)
        # steady state: rounds of [next-chunk scores x8, current PV x8];
        # the final chunk's PV weaves into the second-to-last chunk's rounds
        for ci in range(2, 7):
            for i in range(8):
                chunks[ci + 1].emit_s(4)
                chunks[ci].emit_pv(4)
                if ci == 6 and i >= 2:
                    chunks[7].emit_pv(4)
            epilogue(chunks[ci].t, chunks[ci].qc, chunks[ci].o_ps)
        chunks[7].emit_pv(NBLK)
        epilogue(chunks[7].t, chunks[7].qc, chunks[7].o_ps)

    nc.compile()
    return nc


def _get_nc():
    if "nc" not in _CACHE:
        _CACHE["nc"] = _build()
    return _CACHE["nc"]


def _shard_inputs(q, k, v, mask, Wq, Wk, Wv):
    import ml_dtypes

    bf16 = ml_dtypes.bfloat16
    q = np.asarray(q, np.float32)
    k = np.asarray(k, np.float32)
    v = np.asarray(v, np.float32)
    mask = np.asarray(mask, np.float32)
    Wq = np.asarray(Wq, np.float32)
    Wk = np.asarray(Wk, np.float32)
    Wv = np.asarray(Wv, np.float32)

    ident = np.eye(128, dtype=np.float32)
    qTs = [np.ascontiguousarray(q[b].T).astype(bf16) for b in range(B)]
    kTs = [np.ascontiguousarray(k[b].T).astype(bf16) for b in range(B)]
    vTs = [np.ascontiguousarray(v[b].T).astype(bf16) for b in range(B)]
    msks = [
        np.ascontiguousarray(mask[b].reshape(NKC, 128).T).astype(np.float32)
        for b in range(B)
    ]
    in_maps = []
    for c in range(8):
        b, j = c // 4, c % 4
        sl = slice(j * NCOL, (j + 1) * NCOL)
        in_maps.append(
            {
                "qT": qTs[b],
                "kT": kTs[b],
                "vT": vTs[b],
                "wq": np.ascontiguousarray(Wq[:, sl]).astype(bf16),
                "wk": np.ascontiguousarray(Wk[:, sl]).astype(bf16),
                "wv": np.ascontiguousarray(Wv[:, sl]).astype(bf16),
                "msk": msks[b],
                "ident": ident,
            }
        )
    return in_maps


def _assemble(results):
    """results: list of 8 dicts with 'out' [2*S, 128] -> full [B, S, 1024]."""
    outp = np.empty((B, S, HEADS * DK), np.float32)
    for c in range(8):
        b, j = c // 4, c % 4
        o = np.asarray(results[c]["out"]).reshape(2, S, 128)
        outp[b, :, j * NCOL: j * NCOL + 128] = o[0]
        outp[b, :, j * NCOL + 128: j * NCOL + 256] = o[1]
    return outp


def kernel(q, k, v, mask, Wq, Wk, Wv):
    from concourse.bass_utils import run_bass_kernel_spmd

    nc = _get_nc()
    in_maps = _shard_inputs(q, k, v, mask, Wq, Wk, Wv)
    res = run_bass_kernel_spmd(nc, in_maps, core_ids=list(range(8))).results
    return _assemble(res)


# revision 34
# speedup vs baseline: 1.0458x; 1.0458x over previous
"""Distributed attention kernel for Trainium2 (8 NeuronCores).

Sharding: B*H = 2*16 = 32 (batch, head) pairs over 8 cores.
Core c handles batch b = c//4 and global heads 4*(c%4) .. 4*(c%4)+3
(i.e. output columns (c%4)*256 : (c%4+1)*256 of the 1024-wide output).

Per-core kernel (compute in bf16, f32 PSUM accumulation):
  - inputs arrive pre-transposed from host: qT/kT/vT [1024, 2048] bf16,
    weight column slices wq/wk/wv [1024, 256] bf16, mask columns msk
    [128, 16] f32 (0/1), identity [128, 128] f32.
  - projections: QWT/KWT in [d, s] layout, VW in [s, d] layout. VW gets a
    mask-valued ones column appended per head (65 cols) so the PV matmul
    also produces sum(exp) in its last output row; VW rows for masked keys
    are zeroed, which implements the additive -1e12 key mask exactly.
  - scores computed transposed: S_T[k, q] so softmax needs no P transpose;
    the two heads of a projection tile are row-packed (64x128 PE tiling);
    exp on ScalarE over 3-bank PSUM tiles; PV accumulation into [65, 512].
  - epilogue: TensorE transposes [65,128]->[128,65] (O plus sumexp),
    reciprocal + per-partition scale on VectorE, DMA out (t-major layout,
    host reassembles).
"""

import numpy as np

HEADS = 16
DK = 64
DM = 1024
B = 2
S = 2048
HL = 4           # heads per core
NCOL = HL * DK   # 256 projection cols per core
NM = DM // 128   # 8 m-chunks
NKC = S // 128   # 16 k-chunks
NQC = S // 512   # 4 q-chunks
NSUB = 512 // 128
NBLK = 2 * NKC   # 32 score blocks of 512 per (t, qc); 2 blocks per s-tile
NST = NBLK // 2  # 16 s-tiles per (t, qc)

_CACHE = {}
CFG = {"pack_s": True, "pe_chain": False, "crit": False, "round_edge": False}


def _build(loop_n=None):
    from contextlib import ExitStack, nullcontext
    import concourse.bass as bass  # noqa: F401
    import concourse.mybir as mybir
    import concourse.bacc as bacc
    import concourse.tile as tile

    f32 = mybir.dt.float32
    bf16 = mybir.dt.bfloat16
    Exp = mybir.ActivationFunctionType.Exp

    nc = bacc.Bacc("TRN2", target_bir_lowering=False, debug=False, num_devices=8)

    qT = nc.dram_tensor("qT", [DM, S], bf16, kind="ExternalInput").ap()
    kT = nc.dram_tensor("kT", [DM, S], bf16, kind="ExternalInput").ap()
    vT = nc.dram_tensor("vT", [DM, S], bf16, kind="ExternalInput").ap()
    # weights arrive host-swizzled to the SBUF layout [128, NM*NCOL]
    wq = nc.dram_tensor("wq", [128, NM * NCOL], bf16, kind="ExternalInput").ap()
    wk = nc.dram_tensor("wk", [128, NM * NCOL], bf16, kind="ExternalInput").ap()
    wv = nc.dram_tensor("wv", [128, NM * NCOL], bf16, kind="ExternalInput").ap()
    msk = nc.dram_tensor("msk", [128, NKC], f32, kind="ExternalInput").ap()
    ident = nc.dram_tensor("ident", [128, 128], f32, kind="ExternalInput").ap()
    # t-major output: rows [t*2048 + q], 128 cols (heads 2t, 2t+1)
    out = nc.dram_tensor("out", [2 * S, 128], f32, kind="ExternalOutput").ap()

    with tile.TileContext(nc) as tc, ExitStack() as ctx:
        const = ctx.enter_context(tc.tile_pool(name="const", bufs=1))
        bigp = ctx.enter_context(tc.tile_pool(name="bigp", bufs=2, space="PSUM"))
        op = ctx.enter_context(tc.tile_pool(name="op", bufs=4, space="PSUM"))
        ep = ctx.enter_context(tc.tile_pool(name="ep", bufs=24))
        otsp = ctx.enter_context(tc.tile_pool(name="otsp", bufs=4))
        outp = ctx.enter_context(tc.tile_pool(name="outp", bufs=4))
        rcp = ctx.enter_context(tc.tile_pool(name="rcp", bufs=4))

        # ---- persistent SBUF tensors ----
        xq = const.tile([128, NM * S], bf16, tag="xq")
        xk = const.tile([128, NM * S], bf16, tag="xk")
        xv = const.tile([128, NM * S], bf16, tag="xv")
        wq_sb = const.tile([128, NM * NCOL], bf16, tag="wq")
        wk_sb = const.tile([128, NM * NCOL], bf16, tag="wk")
        wv_sb = const.tile([128, NM * NCOL], bf16, tag="wv")
        m_sb = const.tile([128, NKC], f32, tag="m")
        id_sb = const.tile([128, 128], f32, tag="id")
        qwt = const.tile([128, 2 * S], bf16, tag="qwt")    # [d(2 heads), s] x2
        kwt = const.tile([128, 2 * S], bf16, tag="kwt")
        vw1 = const.tile([128, NKC * HL * 65], bf16, tag="vw1")

        if loop_n:
            # benchmark variant: run the whole body loop_n times on-device
            ctx.enter_context(tc.For_i(0, loop_n, 1))

        # ---- input DMA: weights/consts, then xk/xq interleaved, then xv ----
        for dst, wsrc in ((wk_sb, wk), (wq_sb, wq), (wv_sb, wv)):
            nc.sync.dma_start(out=dst[:, :], in_=wsrc)
        nc.sync.dma_start(out=m_sb[:, :], in_=msk)
        nc.sync.dma_start(out=id_sb[:, :], in_=ident)
        for m in range(NM):
            nc.sync.dma_start(
                out=xk[:, m * S: (m + 1) * S], in_=kT[m * 128: (m + 1) * 128, :]
            )
            nc.sync.dma_start(
                out=xq[:, m * S: (m + 1) * S], in_=qT[m * 128: (m + 1) * 128, :]
            )
        for m in range(NM):
            nc.sync.dma_start(
                out=xv[:, m * S: (m + 1) * S], in_=vT[m * 128: (m + 1) * 128, :]
            )

        # ---- ones columns of vw1 (mask-valued): vw1[:, kc, h, 64] = msk[:, kc]
        vw1_4d = vw1[:, :].rearrange("p (k h c) -> p k h c", k=NKC, h=HL)
        for h in range(HL):
            nc.vector.tensor_copy(vw1_4d[:, :, h, 64], m_sb[:, :])

        from concourse.tile_rust import add_dep_helper

        pe_prev = [None]
        last_s = [None]
        last_pv = [None]

        def pe_dep(bi):
            """Optionally chain TensorE instructions in emission order so the
            scheduler keeps same-PE-mode runs together (mode switches between
            64x128-tiled scores and default matmuls cost a PE drain)."""
            if CFG.get("pe_chain") and pe_prev[0] is not None:
                add_dep_helper(bi.ins, pe_prev[0].ins, sync=False,
                               reason="pe emission order")
            pe_prev[0] = bi
            return bi

        def proj_qk(w_sb, x_sb, dst, t, qc):
            ps = bigp.tile([128, 512], f32, tag="big", name=f"pqk{t}_{qc}")
            for m in range(NM):
                pe_dep(nc.tensor.matmul(
                    ps[:, :],
                    lhsT=w_sb[:, m * NCOL + t * 128: m * NCOL + t * 128 + 128],
                    rhs=x_sb[:, m * S + qc * 512: m * S + qc * 512 + 512],
                    start=(m == 0),
                    stop=(m == NM - 1),
                ))
            nc.vector.tensor_copy(
                dst[:, t * S + qc * 512: t * S + qc * 512 + 512], ps[:, :]
            )

        def proj_qk_mouter(w_sb, x_sb, dst, t):
            # m-outer: consume each x chunk as its DMA lands (4 live tiles)
            ps = [
                bigp.tile([128, 512], f32, tag="big", name=f"pm{t}_{q}")
                for q in (0, 1)
            ] + [
                op.tile([128, 512], f32, tag="o", name=f"pm{t}_{q}")
                for q in (2, 3)
            ]
            for m in range(NM):
                for qc in range(NQC):
                    pe_dep(nc.tensor.matmul(
                        ps[qc][:, :],
                        lhsT=w_sb[:, m * NCOL + t * 128: m * NCOL + t * 128 + 128],
                        rhs=x_sb[:, m * S + qc * 512: m * S + qc * 512 + 512],
                        start=(m == 0),
                        stop=(m == NM - 1),
                    ))
                    if m == NM - 1:
                        nc.vector.tensor_copy(
                            dst[:, t * S + qc * 512: t * S + qc * 512 + 512],
                            ps[qc][:, :],
                        )

        def proj_v(kb):
            ps = bigp.tile([128, NCOL], f32, tag="big", name=f"pv{kb}")
            for m in range(NM):
                pe_dep(nc.tensor.matmul(
                    ps[:, :],
                    lhsT=xv[:, m * S + kb * 128: m * S + kb * 128 + 128],
                    rhs=wv_sb[:, m * NCOL: (m + 1) * NCOL],
                    start=(m == 0),
                    stop=(m == NM - 1),
                ))
            nc.vector.tensor_scalar_mul(
                vw1_4d[:, kb, :, 0:64],
                ps[:, :].rearrange("p (h c) -> p h c", h=HL),
                m_sb[:, kb: kb + 1],
            )

        class Chunk:
            """Incremental emitter for one (t, qc) attention chunk."""

            def __init__(self, t, qc):
                self.t, self.qc = t, qc
                self.s_tiles = [None] * NST
                self.e_tiles = [None] * NST
                self.o_ps = None
                self.si = 0
                self.pi = 0

            def emit_s(self, n):
                t, qc = self.t, self.qc
                todo = list(range(self.si, min(self.si + n, NBLK)))
                if not todo:
                    return
                self.si = todo[-1] + 1
                for g in range(0, len(todo), 4):
                    blks = todo[g: g + 4]
                    for blk in blks:
                        st = blk // 2
                        if blk % 2 == 0:
                            self.s_tiles[st] = bigp.tile(
                                [128, 1024], f32, tag="big",
                                name=f"sps{t}_{qc}_{st}"
                            )
                    crit = tc.tile_critical() if CFG.get("crit") else None
                    if crit:
                        crit.__enter__()
                    for j, blk in enumerate(blks):
                        kc, a = divmod(blk, 2)
                        st, sc = divmod(blk, 2)
                        bi = pe_dep(nc.tensor.matmul(
                            self.s_tiles[st][:, sc * 512: (sc + 1) * 512],
                            lhsT=kwt[
                                64 * a: 64 * a + 64,
                                t * S + kc * 128: t * S + kc * 128 + 128,
                            ],
                            rhs=qwt[
                                64 * a: 64 * a + 64,
                                t * S + qc * 512: t * S + qc * 512 + 512,
                            ],
                            start=True,
                            stop=True,
                            tile_position=(64 * a, 0) if CFG["pack_s"] else None,
                        ))
                        if (CFG.get("round_edge") and j == 0
                                and last_pv[0] is not None):
                            add_dep_helper(bi.ins, last_pv[0].ins, sync=False,
                                           reason="round order s after pv")
                        last_s[0] = bi
                    if crit:
                        crit.__exit__(None, None, None)
                    for blk in blks:
                        st, sc = divmod(blk, 2)
                        if blk == NBLK - 1 or sc == 1:
                            w = (sc + 1) * 512
                            self.e_tiles[st] = ep.tile(
                                [128, 1024], bf16, tag="e",
                                name=f"et{t}_{qc}_{st}"
                            )
                            nc.scalar.activation(
                                self.e_tiles[st][:, 0:w],
                                self.s_tiles[st][:, 0:w],
                                Exp,
                                scale=0.125,
                            )

            def emit_pv(self, n):
                t, qc = self.t, self.qc
                if self.o_ps is None:
                    self.o_ps = [
                        op.tile([65, 512], f32, tag="o", name=f"ops{t}_{qc}_{a}")
                        for a in range(2)
                    ]
                blks = list(range(self.pi, min(self.pi + n, NBLK)))
                if not blks:
                    return
                self.pi = blks[-1] + 1
                crit = tc.tile_critical() if CFG.get("crit") else None
                if crit:
                    crit.__enter__()
                for j, blk in enumerate(blks):
                    kc, a = divmod(blk, 2)
                    st, sc = divmod(blk, 2)
                    bi = pe_dep(nc.tensor.matmul(
                        self.o_ps[a][:, :],
                        lhsT=vw1[
                            :, kc * HL * 65 + (2 * t + a) * 65:
                            kc * HL * 65 + (2 * t + a) * 65 + 65
                        ],
                        rhs=self.e_tiles[st][:, sc * 512: (sc + 1) * 512],
                        start=(kc == 0),
                        stop=(kc == NKC - 1),
                    ))
                    if (CFG.get("round_edge") and j == 0
                            and last_s[0] is not None):
                        add_dep_helper(bi.ins, last_s[0].ins, sync=False,
                                       reason="round order pv after s")
                    last_pv[0] = bi
                if crit:
                    crit.__exit__(None, None, None)

        def epilogue(t, qc, o_ps):
            # copy out, transpose O+sumexp, reciprocal, normalize, DMA out
            ots2 = []
            for a in range(2):
                ots = otsp.tile([65, 512], f32, tag="ots", name=f"ots{t}_{qc}_{a}")
                nc.vector.tensor_copy(ots[:, :], o_ps[a][:, :])
                ots2.append(ots)
            for sub in range(NSUB):
                t2 = op.tile([128, 2 * 65], f32, tag="o", name=f"t2{t}_{qc}_{sub}")
                for a in range(2):
                    pe_dep(nc.tensor.transpose(
                        t2[:, a * 65: (a + 1) * 65],
                        ots2[a][:, sub * 128: (sub + 1) * 128],
                        id_sb[0:65, 0:65],
                    ))
                rc = rcp.tile([128, 2], f32, tag="rc", name=f"rc{t}_{qc}_{sub}")
                t2_3d = t2[:, :].rearrange("p (h c) -> p h c", h=2)
                nc.vector.reciprocal_approx_fast(out=rc[:, :], in_=t2_3d[:, :, 64])
                o_out = outp.tile([128, 128], f32, tag="out", name=f"oo{t}_{qc}_{sub}")
                for a in range(2):
                    nc.vector.tensor_scalar_mul(
                        o_out[:, a * 64: (a + 1) * 64],
                        t2[:, a * 65: a * 65 + 64],
                        rc[:, a: a + 1],
                    )
                nc.sync.dma_start(
                    out=out[
                        t * S + qc * 512 + sub * 128:
                        t * S + qc * 512 + sub * 128 + 128, :
                    ],
                    in_=o_out[:, :],
                )

        # ---- schedule (PE order fully chained via pe_dep):
        # warmup: K/Q t0 projections woven with chunk0 scores; then rounds
        # of [scores x8, other-128-mode work, PV x8] so PE mode switches are
        # amortized over 8-matmul runs and ScalarE never starves.
        chunks = [Chunk(t, qc) for t in range(2) for qc in range(NQC)]

        proj_qk_mouter(wk_sb, xk, kwt, 0)
        proj_qk_mouter(wq_sb, xq, qwt, 0)
        chunks[0].emit_s(NBLK)
        # V projection + chunk1 scores + chunk0 PV
        for i in range(8):
            chunks[1].emit_s(4)
            proj_v(2 * i)
            proj_v(2 * i + 1)
            chunks[0].emit_pv(4)
        epilogue(0, 0, chunks[0].o_ps)
        # t1 projections + chunk2 scores + chunk1 PV
        projt1 = [(wk_sb, xk, kwt), (wq_sb, xq, qwt)]
        for j in range(8):
            w_sb, x_sb, dst = projt1[j // 4]
            chunks[2].emit_s(4)
            proj_qk(w_sb, x_sb, dst, 1, j % 4)
            chunks[1].emit_pv(4)
        epilogue(0, 1, chunks[1].o_ps)
        # steady state: rounds of [next-chunk scores x8, current PV x8];
        # the final chunk's PV weaves into the second-to-last chunk's rounds
        for ci in range(2, 7):
            for i in range(8):
                chunks[ci + 1].emit_s(4)
                chunks[ci].emit_pv(4)
                if ci == 6 and i >= 2:
                    chunks[7].emit_pv(4)
            epilogue(chunks[ci].t, chunks[ci].qc, chunks[ci].o_ps)
        chunks[7].emit_pv(NBLK)
        epilogue(chunks[7].t, chunks[7].qc, chunks[7].o_ps)

    nc.compile()
    return nc


def _get_nc():
    if "nc" not in _CACHE:
        _CACHE["nc"] = _build()
    return _CACHE["nc"]


def _shard_inputs(q, k, v, mask, Wq, Wk, Wv):
    import ml_dtypes

    bf16 = ml_dtypes.bfloat16
    q = np.asarray(q, np.float32)
    k = np.asarray(k, np.float32)
    v = np.asarray(v, np.float32)
    mask = np.asarray(mask, np.float32)
    Wq = np.asarray(Wq, np.float32)
    Wk = np.asarray(Wk, np.float32)
    Wv = np.asarray(Wv, np.float32)

    def _swz(w):
        # [1024, 256] -> SBUF layout [128, 8*256] (row p = concat_m W[m*128+p])
        return np.ascontiguousarray(
            w.reshape(NM, 128, NCOL).transpose(1, 0, 2).reshape(128, NM * NCOL)
        ).astype(bf16)

    ident = np.eye(128, dtype=np.float32)
    qTs = [np.ascontiguousarray(q[b].T).astype(bf16) for b in range(B)]
    kTs = [np.ascontiguousarray(k[b].T).astype(bf16) for b in range(B)]
    vTs = [np.ascontiguousarray(v[b].T).astype(bf16) for b in range(B)]
    msks = [
        np.ascontiguousarray(mask[b].reshape(NKC, 128).T).astype(np.float32)
        for b in range(B)
    ]
    in_maps = []
    for c in range(8):
        b, j = c // 4, c % 4
        sl = slice(j * NCOL, (j + 1) * NCOL)
        in_maps.append(
            {
                "qT": qTs[b],
                "kT": kTs[b],
                "vT": vTs[b],
                "wq": _swz(Wq[:, sl]),
                "wk": _swz(Wk[:, sl]),
                "wv": _swz(Wv[:, sl]),
                "msk": msks[b],
                "ident": ident,
            }
        )
    return in_maps


def _assemble(results):
    """results: list of 8 dicts with 'out' [2*S, 128] -> full [B, S, 1024]."""
    outp = np.empty((B, S, HEADS * DK), np.float32)
    for c in range(8):
        b, j = c // 4, c % 4
        o = np.asarray(results[c]["out"]).reshape(2, S, 128)
        outp[b, :, j * NCOL: j * NCOL + 128] = o[0]
        outp[b, :, j * NCOL + 128: j * NCOL + 256] = o[1]
    return outp


def kernel(q, k, v, mask, Wq, Wk, Wv):
    from concourse.bass_utils import run_bass_kernel_spmd

    nc = _get_nc()
    in_maps = _shard_inputs(q, k, v, mask, Wq, Wk, Wv)
    res = run_bass_kernel_spmd(nc, in_maps, core_ids=list(range(8))).results
    return _assemble(res)


# revision 35
# speedup vs baseline: 1.0719x; 1.0249x over previous
"""Distributed attention kernel for Trainium2 (8 NeuronCores).

Sharding: B*H = 2*16 = 32 (batch, head) pairs over 8 cores.
Core c handles batch b = c//4 and global heads 4*(c%4) .. 4*(c%4)+3
(i.e. output columns (c%4)*256 : (c%4+1)*256 of the 1024-wide output).

Per-core kernel (compute in bf16, f32 PSUM accumulation):
  - inputs arrive pre-transposed from host: qT/kT/vT [1024, 2048] bf16,
    weight column slices wq/wk/wv [1024, 256] bf16, mask columns msk
    [128, 16] f32 (0/1), identity [128, 128] f32.
  - projections: QWT/KWT in [d, s] layout, VW in [s, d] layout. VW gets a
    mask-valued ones column appended per head (65 cols) so the PV matmul
    also produces sum(exp) in its last output row; VW rows for masked keys
    are zeroed, which implements the additive -1e12 key mask exactly.
  - scores computed transposed: S_T[k, q] so softmax needs no P transpose;
    the two heads of a projection tile are row-packed (64x128 PE tiling);
    exp on ScalarE over 3-bank PSUM tiles; PV accumulation into [65, 512].
  - epilogue: TensorE transposes [65,128]->[128,65] (O plus sumexp),
    reciprocal + per-partition scale on VectorE, DMA out (t-major layout,
    host reassembles).
"""

import numpy as np

HEADS = 16
DK = 64
DM = 1024
B = 2
S = 2048
HL = 4           # heads per core
NCOL = HL * DK   # 256 projection cols per core
NM = DM // 128   # 8 m-chunks
NKC = S // 128   # 16 k-chunks
NQC = S // 512   # 4 q-chunks
NSUB = 512 // 128
NBLK = 2 * NKC   # 32 score blocks of 512 per (t, qc); 2 blocks per s-tile
NST = NBLK // 2  # 16 s-tiles per (t, qc)

_CACHE = {}
CFG = {"pack_s": True, "pe_chain": False, "crit": False, "round_edge": False}


def _build(loop_n=None):
    from contextlib import ExitStack, nullcontext
    import concourse.bass as bass  # noqa: F401
    import concourse.mybir as mybir
    import concourse.bacc as bacc
    import concourse.tile as tile

    f32 = mybir.dt.float32
    bf16 = mybir.dt.bfloat16
    Exp = mybir.ActivationFunctionType.Exp

    nc = bacc.Bacc("TRN2", target_bir_lowering=False, debug=False, num_devices=8)

    qT = nc.dram_tensor("qT", [DM, S], bf16, kind="ExternalInput").ap()
    kT = nc.dram_tensor("kT", [DM, S], bf16, kind="ExternalInput").ap()
    vT = nc.dram_tensor("vT", [DM, S], bf16, kind="ExternalInput").ap()
    # weights arrive host-swizzled to the SBUF layout [128, NM*NCOL]
    wq = nc.dram_tensor("wq", [128, NM * NCOL], bf16, kind="ExternalInput").ap()
    wk = nc.dram_tensor("wk", [128, NM * NCOL], bf16, kind="ExternalInput").ap()
    wv = nc.dram_tensor("wv", [128, NM * NCOL], bf16, kind="ExternalInput").ap()
    msk = nc.dram_tensor("msk", [128, NKC], f32, kind="ExternalInput").ap()
    ident = nc.dram_tensor("ident", [128, 128], f32, kind="ExternalInput").ap()
    # t-major output: rows [t*2048 + q], 128 cols (heads 2t, 2t+1)
    out = nc.dram_tensor("out", [2 * S, 128], f32, kind="ExternalOutput").ap()

    with tile.TileContext(nc) as tc, ExitStack() as ctx:
        const = ctx.enter_context(tc.tile_pool(name="const", bufs=1))
        bigp = ctx.enter_context(tc.tile_pool(name="bigp", bufs=2, space="PSUM"))
        op = ctx.enter_context(tc.tile_pool(name="op", bufs=4, space="PSUM"))
        ep = ctx.enter_context(tc.tile_pool(name="ep", bufs=24))
        otsp = ctx.enter_context(tc.tile_pool(name="otsp", bufs=4))
        outp = ctx.enter_context(tc.tile_pool(name="outp", bufs=4))
        rcp = ctx.enter_context(tc.tile_pool(name="rcp", bufs=4))

        # ---- persistent SBUF tensors ----
        xq = const.tile([128, NM * S], bf16, tag="xq")
        xk = const.tile([128, NM * S], bf16, tag="xk")
        xv = const.tile([128, NM * S], bf16, tag="xv")
        wq_sb = const.tile([128, NM * NCOL], bf16, tag="wq")
        wk_sb = const.tile([128, NM * NCOL], bf16, tag="wk")
        wv_sb = const.tile([128, NM * NCOL], bf16, tag="wv")
        m_sb = const.tile([128, NKC], f32, tag="m")
        id_sb = const.tile([128, 128], f32, tag="id")
        qwt = const.tile([128, 2 * S], bf16, tag="qwt")    # [d(2 heads), s] x2
        kwt = const.tile([128, 2 * S], bf16, tag="kwt")
        vw1 = const.tile([128, NKC * HL * 65], bf16, tag="vw1")

        if loop_n:
            # benchmark variant: run the whole body loop_n times on-device
            ctx.enter_context(tc.For_i(0, loop_n, 1))

        # ---- input DMA: weights/consts, then xk/xq interleaved, then xv ----
        for dst, wsrc in ((wk_sb, wk), (wq_sb, wq), (wv_sb, wv)):
            nc.sync.dma_start(out=dst[:, :], in_=wsrc)
        nc.sync.dma_start(out=m_sb[:, :], in_=msk)
        nc.sync.dma_start(out=id_sb[:, :], in_=ident)
        for dst, src_ in ((xk, kT), (xq, qT), (xv, vT)):
            for m in range(NM):
                nc.sync.dma_start(
                    out=dst[:, m * S: (m + 1) * S],
                    in_=src_[m * 128: (m + 1) * 128, :],
                )

        # ---- ones columns of vw1 (mask-valued): vw1[:, kc, h, 64] = msk[:, kc]
        vw1_4d = vw1[:, :].rearrange("p (k h c) -> p k h c", k=NKC, h=HL)
        for h in range(HL):
            nc.vector.tensor_copy(vw1_4d[:, :, h, 64], m_sb[:, :])

        from concourse.tile_rust import add_dep_helper

        pe_prev = [None]
        last_s = [None]
        last_pv = [None]

        def pe_dep(bi):
            """Optionally chain TensorE instructions in emission order so the
            scheduler keeps same-PE-mode runs together (mode switches between
            64x128-tiled scores and default matmuls cost a PE drain)."""
            if CFG.get("pe_chain") and pe_prev[0] is not None:
                add_dep_helper(bi.ins, pe_prev[0].ins, sync=False,
                               reason="pe emission order")
            pe_prev[0] = bi
            return bi

        def proj_qk(w_sb, x_sb, dst, t, qc):
            ps = bigp.tile([128, 512], f32, tag="big", name=f"pqk{t}_{qc}")
            for m in range(NM):
                pe_dep(nc.tensor.matmul(
                    ps[:, :],
                    lhsT=w_sb[:, m * NCOL + t * 128: m * NCOL + t * 128 + 128],
                    rhs=x_sb[:, m * S + qc * 512: m * S + qc * 512 + 512],
                    start=(m == 0),
                    stop=(m == NM - 1),
                ))
            nc.vector.tensor_copy(
                dst[:, t * S + qc * 512: t * S + qc * 512 + 512], ps[:, :]
            )

        def proj_qk_mouter(w_sb, x_sb, dst, t):
            # m-outer: consume each x chunk as its DMA lands (4 live tiles)
            ps = [
                bigp.tile([128, 512], f32, tag="big", name=f"pm{t}_{q}")
                for q in (0, 1)
            ] + [
                op.tile([128, 512], f32, tag="o", name=f"pm{t}_{q}")
                for q in (2, 3)
            ]
            for m in range(NM):
                for qc in range(NQC):
                    pe_dep(nc.tensor.matmul(
                        ps[qc][:, :],
                        lhsT=w_sb[:, m * NCOL + t * 128: m * NCOL + t * 128 + 128],
                        rhs=x_sb[:, m * S + qc * 512: m * S + qc * 512 + 512],
                        start=(m == 0),
                        stop=(m == NM - 1),
                    ))
                    if m == NM - 1:
                        nc.vector.tensor_copy(
                            dst[:, t * S + qc * 512: t * S + qc * 512 + 512],
                            ps[qc][:, :],
                        )

        def proj_v(kb):
            ps = bigp.tile([128, NCOL], f32, tag="big", name=f"pv{kb}")
            for m in range(NM):
                pe_dep(nc.tensor.matmul(
                    ps[:, :],
                    lhsT=xv[:, m * S + kb * 128: m * S + kb * 128 + 128],
                    rhs=wv_sb[:, m * NCOL: (m + 1) * NCOL],
                    start=(m == 0),
                    stop=(m == NM - 1),
                ))
            nc.vector.tensor_scalar_mul(
                vw1_4d[:, kb, :, 0:64],
                ps[:, :].rearrange("p (h c) -> p h c", h=HL),
                m_sb[:, kb: kb + 1],
            )

        class Chunk:
            """Incremental emitter for one (t, qc) attention chunk."""

            def __init__(self, t, qc):
                self.t, self.qc = t, qc
                self.s_tiles = [None] * NST
                self.e_tiles = [None] * NST
                self.o_ps = None
                self.si = 0
                self.pi = 0

            def emit_s(self, n):
                t, qc = self.t, self.qc
                todo = list(range(self.si, min(self.si + n, NBLK)))
                if not todo:
                    return
                self.si = todo[-1] + 1
                for g in range(0, len(todo), 4):
                    blks = todo[g: g + 4]
                    for blk in blks:
                        st = blk // 2
                        if blk % 2 == 0:
                            self.s_tiles[st] = bigp.tile(
                                [128, 1024], f32, tag="big",
                                name=f"sps{t}_{qc}_{st}"
                            )
                    crit = tc.tile_critical() if CFG.get("crit") else None
                    if crit:
                        crit.__enter__()
                    for j, blk in enumerate(blks):
                        kc, a = divmod(blk, 2)
                        st, sc = divmod(blk, 2)
                        bi = pe_dep(nc.tensor.matmul(
                            self.s_tiles[st][:, sc * 512: (sc + 1) * 512],
                            lhsT=kwt[
                                64 * a: 64 * a + 64,
                                t * S + kc * 128: t * S + kc * 128 + 128,
                            ],
                            rhs=qwt[
                                64 * a: 64 * a + 64,
                                t * S + qc * 512: t * S + qc * 512 + 512,
                            ],
                            start=True,
                            stop=True,
                            tile_position=(64 * a, 0) if CFG["pack_s"] else None,
                        ))
                        if (CFG.get("round_edge") and j == 0
                                and last_pv[0] is not None):
                            add_dep_helper(bi.ins, last_pv[0].ins, sync=False,
                                           reason="round order s after pv")
                        last_s[0] = bi
                    if crit:
                        crit.__exit__(None, None, None)
                    for blk in blks:
                        st, sc = divmod(blk, 2)
                        if blk == NBLK - 1 or sc == 1:
                            w = (sc + 1) * 512
                            self.e_tiles[st] = ep.tile(
                                [128, 1024], bf16, tag="e",
                                name=f"et{t}_{qc}_{st}"
                            )
                            nc.scalar.activation(
                                self.e_tiles[st][:, 0:w],
                                self.s_tiles[st][:, 0:w],
                                Exp,
                                scale=0.125,
                            )

            def emit_pv(self, n):
                t, qc = self.t, self.qc
                if self.o_ps is None:
                    self.o_ps = [
                        op.tile([65, 512], f32, tag="o", name=f"ops{t}_{qc}_{a}")
                        for a in range(2)
                    ]
                blks = list(range(self.pi, min(self.pi + n, NBLK)))
                if not blks:
                    return
                self.pi = blks[-1] + 1
                crit = tc.tile_critical() if CFG.get("crit") else None
                if crit:
                    crit.__enter__()
                for j, blk in enumerate(blks):
                    kc, a = divmod(blk, 2)
                    st, sc = divmod(blk, 2)
                    bi = pe_dep(nc.tensor.matmul(
                        self.o_ps[a][:, :],
                        lhsT=vw1[
                            :, kc * HL * 65 + (2 * t + a) * 65:
                            kc * HL * 65 + (2 * t + a) * 65 + 65
                        ],
                        rhs=self.e_tiles[st][:, sc * 512: (sc + 1) * 512],
                        start=(kc == 0),
                        stop=(kc == NKC - 1),
                    ))
                    if (CFG.get("round_edge") and j == 0
                            and last_s[0] is not None):
                        add_dep_helper(bi.ins, last_s[0].ins, sync=False,
                                       reason="round order pv after s")
                    last_pv[0] = bi
                if crit:
                    crit.__exit__(None, None, None)

        def epilogue(t, qc, o_ps):
            # copy out, transpose O+sumexp, reciprocal, normalize, DMA out
            ots2 = []
            for a in range(2):
                ots = otsp.tile([65, 512], f32, tag="ots", name=f"ots{t}_{qc}_{a}")
                nc.vector.tensor_copy(ots[:, :], o_ps[a][:, :])
                ots2.append(ots)
            for sub in range(NSUB):
                t2 = op.tile([128, 2 * 65], f32, tag="o", name=f"t2{t}_{qc}_{sub}")
                for a in range(2):
                    pe_dep(nc.tensor.transpose(
                        t2[:, a * 65: (a + 1) * 65],
                        ots2[a][:, sub * 128: (sub + 1) * 128],
                        id_sb[0:65, 0:65],
                    ))
                rc = rcp.tile([128, 2], f32, tag="rc", name=f"rc{t}_{qc}_{sub}")
                t2_3d = t2[:, :].rearrange("p (h c) -> p h c", h=2)
                nc.vector.reciprocal_approx_fast(out=rc[:, :], in_=t2_3d[:, :, 64])
                o_out = outp.tile([128, 128], f32, tag="out", name=f"oo{t}_{qc}_{sub}")
                for a in range(2):
                    nc.vector.tensor_scalar_mul(
                        o_out[:, a * 64: (a + 1) * 64],
                        t2[:, a * 65: a * 65 + 64],
                        rc[:, a: a + 1],
                    )
                nc.sync.dma_start(
                    out=out[
                        t * S + qc * 512 + sub * 128:
                        t * S + qc * 512 + sub * 128 + 128, :
                    ],
                    in_=o_out[:, :],
                )

        # ---- schedule (PE order fully chained via pe_dep):
        # warmup: K/Q t0 projections woven with chunk0 scores; then rounds
        # of [scores x8, other-128-mode work, PV x8] so PE mode switches are
        # amortized over 8-matmul runs and ScalarE never starves.
        chunks = [Chunk(t, qc) for t in range(2) for qc in range(NQC)]

        proj_qk_mouter(wk_sb, xk, kwt, 0)
        for qc in range(NQC):
            proj_qk(wk_sb, xk, kwt, 1, qc)   # runs while xq still arriving
        proj_qk_mouter(wq_sb, xq, qwt, 0)
        chunks[0].emit_s(NBLK)
        # V projection + chunk1 scores + chunk0 PV
        for i in range(8):
            chunks[1].emit_s(4)
            proj_v(2 * i)
            proj_v(2 * i + 1)
            chunks[0].emit_pv(4)
        epilogue(0, 0, chunks[0].o_ps)
        # t1 projections + chunk2 scores + chunk1 PV
        projt1 = [(wk_sb, xk, kwt), (wq_sb, xq, qwt)]
        for j in range(8):
            chunks[2].emit_s(4)
            if j % 2 == 0:
                proj_qk(wq_sb, xq, qwt, 1, j // 2)
            chunks[1].emit_pv(4)
        epilogue(0, 1, chunks[1].o_ps)
        # steady state: rounds of [next-chunk scores x8, current PV x8];
        # the final chunk's PV weaves into the second-to-last chunk's rounds
        for ci in range(2, 7):
            for i in range(8):
                chunks[ci + 1].emit_s(4)
                chunks[ci].emit_pv(4)
                if ci == 6 and i >= 2:
                    chunks[7].emit_pv(4)
            epilogue(chunks[ci].t, chunks[ci].qc, chunks[ci].o_ps)
        chunks[7].emit_pv(NBLK)
        epilogue(chunks[7].t, chunks[7].qc, chunks[7].o_ps)

    nc.compile()
    return nc


def _get_nc():
    if "nc" not in _CACHE:
        _CACHE["nc"] = _build()
    return _CACHE["nc"]


def _shard_inputs(q, k, v, mask, Wq, Wk, Wv):
    import ml_dtypes

    bf16 = ml_dtypes.bfloat16
    q = np.asarray(q, np.float32)
    k = np.asarray(k, np.float32)
    v = np.asarray(v, np.float32)
    mask = np.asarray(mask, np.float32)
    Wq = np.asarray(Wq, np.float32)
    Wk = np.asarray(Wk, np.float32)
    Wv = np.asarray(Wv, np.float32)

    def _swz(w):
        # [1024, 256] -> SBUF layout [128, 8*256] (row p = concat_m W[m*128+p])
        return np.ascontiguousarray(
            w.reshape(NM, 128, NCOL).transpose(1, 0, 2).reshape(128, NM * NCOL)
        ).astype(bf16)

    ident = np.eye(128, dtype=np.float32)
    qTs = [np.ascontiguousarray(q[b].T).astype(bf16) for b in range(B)]
    kTs = [np.ascontiguousarray(k[b].T).astype(bf16) for b in range(B)]
    vTs = [np.ascontiguousarray(v[b].T).astype(bf16) for b in range(B)]
    msks = [
        np.ascontiguousarray(mask[b].reshape(NKC, 128).T).astype(np.float32)
        for b in range(B)
    ]
    in_maps = []
    for c in range(8):
        b, j = c // 4, c % 4
        sl = slice(j * NCOL, (j + 1) * NCOL)
        in_maps.append(
            {
                "qT": qTs[b],
                "kT": kTs[b],
                "vT": vTs[b],
                "wq": _swz(Wq[:, sl]),
                "wk": _swz(Wk[:, sl]),
                "wv": _swz(Wv[:, sl]),
                "msk": msks[b],
                "ident": ident,
            }
        )
    return in_maps


def _assemble(results):
    """results: list of 8 dicts with 'out' [2*S, 128] -> full [B, S, 1024]."""
    outp = np.empty((B, S, HEADS * DK), np.float32)
    for c in range(8):
        b, j = c // 4, c % 4
        o = np.asarray(results[c]["out"]).reshape(2, S, 128)
        outp[b, :, j * NCOL: j * NCOL + 128] = o[0]
        outp[b, :, j * NCOL + 128: j * NCOL + 256] = o[1]
    return outp


def kernel(q, k, v, mask, Wq, Wk, Wv):
    from concourse.bass_utils import run_bass_kernel_spmd

    nc = _get_nc()
    in_maps = _shard_inputs(q, k, v, mask, Wq, Wk, Wv)
    res = run_bass_kernel_spmd(nc, in_maps, core_ids=list(range(8))).results
    return _assemble(res)
